# revision 46
# baseline (speedup 1.0000x reference)
"""Trainium2 Bass kernel for the DialogGCN GAT-style message-passing layer.

Math notes (why this is much cheaper than the reference graph):
  Kp    = concat(K, kfeat) @ Wk + bk                    (B,N,D)
  alpha = Q@wden[:D] + Kp@wden[D:] + bden               (B,N)
  w     = softmax(alpha - (1-adj)*1e30, axis=N)
  out   = sum_n w * ((Kp@Wr0)*sm + (Kp@Wr1)*(1-sm))

* softmax is invariant to per-row constants, so the Q term, bden and the
  bk@wden[D:] constant all cancel:  w = softmax_n(X_n . v) masked, where
  X = concat(K, kfeat) and v = Wk @ wden[D:]  (folded on host).
* the output is linear in the weighted sums:
    out = (sum_n w*sm*X_n | c0) @ [Wk;bk] @ Wr0 + (sum_n w*(1-sm)*X_n | c1) @ [Wk;bk] @ Wr1
  so G0 = [Wk;bk]@Wr0 and G1 = [Wk;bk]@Wr1 are folded on host (769x512 each).
* v is folded INTO the streamed tensor on host: X' = X * v (columnwise) and
  G' = G / v (rowwise) — exact algebra. The device then computes
    s_n = rowsum(X'_n) ; p_n = exp(s_n) ; U0 = sum p*m0*[X'|1] ; U1 = ...
  followed by a tiny projection (U0@G0' + U1@G1') / P, with m0 = adj*sm,
  m1 = adj*(1-sm), P = row 768 of U (the ones column of X'; the ones column
  also shifts every score by +1, which softmax cancels).

Device-side layout tricks:
* X' is uploaded as ONE bf16 tensor [BL, N, 772] = [K*v | k1*v | 1.0 | 0 0 0]
  (772 keeps every 128-token chunk 4B/8B aligned). This halves HBM traffic,
  turns the score pass into a single DVE tensor_reduce per batch (row sums,
  eligible for packed 2x/4x modes), and the ones column makes the softmax
  denominator fall out of the same PE accumulation that computes U.
* masks and the G projection matrices are pre-swizzled to their SBUF layouts
  on host and uploaded bf16, so every DMA is a dtype-preserving HWDGE
  transfer with contiguous per-partition descriptors.

Sharding: pure data parallel over batch B=32 across 8 cores (4 rows each).
"""

import os
import sys

import numpy as np

for _p in ("/opt/trn_rl_repo", "/root/.axon_site/_ro/trn_rl_repo"):
    if os.path.isdir(_p) and _p not in sys.path:
        sys.path.insert(0, _p)

B, N, D, KD = 32, 2048, 512, 256
F = D + KD  # 768
XW = F + 4  # 772: [K | k1 | 1 | 0 0 0] -- pad keeps chunk offsets 8B aligned
NCORES = 8
BL = B // NCORES  # 4 batch rows per core
NT = 16  # free-dim token tiles per batch (N = 128 * NT)

_BUILD_CACHE = {}
last_results = None  # BassKernelResults of the most recent run (for test.py)


def _build():
    """Trace the Bass program (same NEFF runs SPMD on all 8 cores)."""
    import concourse.bass as bass
    import concourse.tile as tile
    from concourse import bacc, mybir
    from concourse.masks import make_identity

    f32 = mybir.dt.float32
    bf16 = mybir.dt.bfloat16

    nc = bacc.Bacc()

    # ---- DRAM I/O ----------------------------------------------------------
    x_f = nc.dram_tensor("x_f", [BL, N, XW], bf16, kind="ExternalInput")
    x_b = nc.dram_tensor("x_b", [BL, N, XW], bf16, kind="ExternalInput")
    # masks pre-swizzled: [partition, branch, m0/m1, batch, half, n]
    mks = nc.dram_tensor("mks", [128, 2, 2, BL, 2, NT // 2], bf16, kind="ExternalInput")
    # G matrices pre-swizzled: [partition, (G0,G1), chunk, D] per branch
    gpkf = nc.dram_tensor("gpkf", [128, 2, 6, D], bf16, kind="ExternalInput")
    gpkb = nc.dram_tensor("gpkb", [128, 2, 6, D], bf16, kind="ExternalInput")
    # row 768 of each G (the bias row)
    g768 = nc.dram_tensor("g768", [1, 4, D], bf16, kind="ExternalInput")
    out_f = nc.dram_tensor("out_f", [BL, D], f32, kind="ExternalOutput")
    out_b = nc.dram_tensor("out_b", [BL, D], f32, kind="ExternalOutput")

    with tile.TileContext(nc) as tc:
        with (
            tc.tile_pool(name="singles", bufs=1) as singles,
            tc.tile_pool(name="xp", bufs=8) as xp,
            tc.tile_pool(name="scp", bufs=3) as scp,
            tc.tile_pool(name="ppp", bufs=6) as ppp,
            tc.tile_pool(name="finp", bufs=2) as finp,
            tc.tile_pool(name="psA", bufs=2, space="PSUM") as psA,
            tc.tile_pool(name="psB", bufs=2, space="PSUM") as psB,
            tc.tile_pool(name="psTr", bufs=2, space="PSUM") as psTr,
            tc.tile_pool(name="psOut", bufs=1, space="PSUM") as psOut,
        ):
            # ---- X loads, one DMA per half-batch (1.58 MB) so scoring can
            # overlap each batch's own transfer; first 8 issued upfront, the
            # rest as compute iterations free their buffers
            NH = NT // 2  # 8 chunks per half
            NGH = 2 * BL * 2  # 16 half-batches
            xsrcs = (x_f, x_b)
            xtiles = {}

            def emit_xdma(gh):
                ibr, rem = divmod(gh, 2 * BL)
                b, h = divmod(rem, 2)
                x = xp.tile([128, NH, XW], bf16, tag="x")
                nc.sync.dma_start(
                    out=x,
                    in_=xsrcs[ibr][b, h * 1024 : (h + 1) * 1024].rearrange(
                        "(p n) d -> p n d", n=NH
                    ),
                )
                xtiles[gh] = x

            for gh in range(8):
                emit_xdma(gh)

            # ---- one-time setup -------------------------------------------
            ident = singles.tile([8, 8], f32)
            make_identity(nc, ident)
            ones11 = singles.tile([1, 1], bf16)
            nc.vector.memset(ones11, 1.0)
            negone = singles.tile([128, 1], f32)
            nc.vector.memset(negone, -1.0)

            mkt = singles.tile([128, 2, 2, BL, 2, NT // 2], bf16)
            nc.scalar.dma_start(out=mkt, in_=mks[:, :, :, :, :, :])
            # branch-f G loads early; branch-b G is only consumed at the very
            # end, so its DMA is issued AFTER the last X transfer (below) to
            # keep it out of the X stream's HBM bandwidth
            gtf = singles.tile([128, 2, 6, D], bf16)
            nc.scalar.dma_start(out=gtf, in_=gpkf[:, :, :, :])
            gtb = singles.tile([128, 2, 6, D], bf16)
            g768t = singles.tile([1, 4, D], bf16)
            nc.scalar.dma_start(out=g768t, in_=g768[:, :, :])

            # ---- streaming + finishing per branch -------------------------
            for ibr, (xsrc, osrc) in enumerate(((x_f, out_f), (x_b, out_b))):
                psAt = psA.tile([8, D], f32)       # rows 0-3: U0(b) K-part, 4-7: U1(b)
                psBt = psB.tile([8, KD + 4], f32)  # cols 0:KD k1-part, col KD = P, pad

                for b in range(BL):
                  for h in range(2):
                    gh = ibr * 2 * BL + b * 2 + h
                    x = xtiles[gh]
                    NA = 3  # chunks of this half scored on ACT; rest on DVE

                    # scores, split across the idle ACT engine (one Copy-accum
                    # per chunk) and the DVE (one batched reduce)
                    sA0 = scp.tile([128, NA], f32, tag="sA0")
                    scr = scp.tile([128, XW], bf16, tag="scr")
                    for n in range(NA):
                        nc.scalar.activation(
                            out=scr,
                            in_=x[:, n, :],
                            func=mybir.ActivationFunctionType.Copy,
                            accum_out=sA0[:, n : n + 1],
                        )
                    sA1 = scp.tile([128, NH - NA], f32, tag="sA1")
                    nc.vector.tensor_reduce(
                        out=sA1,
                        in_=x[:, NA:NH, :],
                        axis=mybir.AxisListType.X,
                        op=mybir.AluOpType.add,
                    )
                    p0 = scp.tile([128, NA], bf16, tag="p0")
                    p1 = scp.tile([128, NH - NA], bf16, tag="p1")
                    # bias=-1 removes the constant from the ones column
                    nc.scalar.activation(
                        out=p0, in_=sA0, func=mybir.ActivationFunctionType.Exp,
                        bias=negone,
                    )
                    nc.scalar.activation(
                        out=p1, in_=sA1, func=mybir.ActivationFunctionType.Exp,
                        bias=negone,
                    )

                    # pp[:, n, :]: col b = p*m0, col 4+b = p*m1, rest 0.
                    # Two tiles (one per score group) so the ACT-scored
                    # chunks' matmuls need not wait for the DVE reduce.
                    pps = []
                    for ph, lo, hi in ((p0, 0, NA), (p1, NA, NH)):
                        ppx = ppp.tile([128, hi - lo, 8], bf16, tag="pp")
                        nc.gpsimd.memset(ppx, 0.0)
                        nc.gpsimd.tensor_mul(
                            ppx[:, :, b], ph, mkt[:, ibr, 0, b, h, lo:hi]
                        )
                        nc.gpsimd.tensor_mul(
                            ppx[:, :, 4 + b], ph, mkt[:, ibr, 1, b, h, lo:hi]
                        )
                        pps.append(ppx)

                    for n in range(NH):
                        first = gh % (2 * BL) == 0 and n == 0
                        last = gh % (2 * BL) == 2 * BL - 1 and n == NH - 1
                        ppt = pps[0] if n < NA else pps[1]
                        nn = n if n < NA else n - NA
                        nc.tensor.matmul(
                            psAt, ppt[:, nn, :], x[:, n, 0:D], start=first, stop=last
                        )
                        nc.tensor.matmul(
                            psBt, ppt[:, nn, :], x[:, n, D:XW], start=first, stop=last
                        )

                    if gh + 8 < NGH:
                        emit_xdma(gh + 8)
                        if gh + 8 == NGH - 1:
                            # last X issued: queue the branch-b G load behind it
                            nc.sync.dma_start(out=gtb, in_=gpkb[:, :, :, :])

                # ---- finishing: out = (U0@G0 + U1@G1) / P ------------------
                uall = finp.tile([8, F + 1], f32, tag="uall")
                nc.vector.tensor_copy(uall[:, 0:D], psAt)
                nc.vector.tensor_copy(uall[:, D : F + 1], psBt[:, 0 : KD + 1])

                uallT = finp.tile([128, 7, 8], f32, tag="uallT")
                for k in range(6):
                    trp = psTr.tile([128, 8], f32, tag="trp")
                    nc.tensor.transpose(trp, uall[:, k * 128 : (k + 1) * 128], ident)
                    nc.vector.tensor_copy(uallT[:, k, :], trp)
                trp = psTr.tile([128, 8], f32, tag="trp")
                nc.tensor.transpose(trp[0:1, :], uall[:, F : F + 1], ident)
                nc.vector.tensor_copy(uallT[0:1, 6, :], trp[0:1, :])
                uTb = finp.tile([128, 7, 8], bf16, tag="uTb")
                nc.vector.tensor_copy(uTb, uallT)

                po = psOut.tile([4, D + 1], f32)  # cols 0:D main, col D = P (bank 2)
                gt = gtf if ibr == 0 else gtb
                for k in range(6):
                    nc.tensor.matmul(
                        po[:, 0:D], uTb[:, k, 0:4], gt[:, 0, k, :],
                        start=(k == 0), stop=False,
                    )
                nc.tensor.matmul(
                    po[:, 0:D], uTb[0:1, 6, 0:4], g768t[0:1, 2 * ibr, :],
                    start=False, stop=False,
                )
                for k in range(6):
                    nc.tensor.matmul(
                        po[:, 0:D], uTb[:, k, 4:8], gt[:, 1, k, :],
                        start=False, stop=False,
                    )
                nc.tensor.matmul(
                    po[:, 0:D], uTb[0:1, 6, 4:8], g768t[0:1, 2 * ibr + 1, :],
                    start=False, stop=True,
                )
                nc.tensor.matmul(
                    po[:, D : D + 1], uTb[0:1, 6, 0:4], ones11, start=True, stop=False
                )
                nc.tensor.matmul(
                    po[:, D : D + 1], uTb[0:1, 6, 4:8], ones11, start=False, stop=True
                )

                rp = finp.tile([4, 1], f32, tag="rp")
                nc.vector.reciprocal(rp, po[:, D : D + 1])
                osb = finp.tile([4, D], f32, tag="osb")
                nc.vector.tensor_scalar_mul(out=osb, in0=po[:, 0:D], scalar1=rp)
                nc.sync.dma_start(out=osrc[:, :], in_=osb)

    nc.compile()
    return nc


def _get_nc():
    if "nc" not in _BUILD_CACHE:
        _BUILD_CACHE["nc"] = _build()
    return _BUILD_CACHE["nc"]


def kernel(**inputs) -> tuple:
    global last_results
    from concourse import mybir
    from concourse.bass_utils import run_bass_kernel_spmd

    f32 = np.float32
    bf16 = np.dtype(mybir.dt.np(mybir.dt.bfloat16))

    K = np.asarray(inputs["K"], dtype=f32)
    front_k1 = np.asarray(inputs["front_k1"], dtype=f32)
    back_K = np.asarray(inputs["back_K"], dtype=f32)
    back_k2 = np.asarray(inputs["back_k2"], dtype=f32)
    Wfk = np.asarray(inputs["Wfk"], dtype=f32)
    bfk = np.asarray(inputs["bfk"], dtype=f32)
    Wbk = np.asarray(inputs["Wbk"], dtype=f32)
    bbk = np.asarray(inputs["bbk"], dtype=f32)
    Wr0 = np.asarray(inputs["Wr0"], dtype=f32)
    Wr1 = np.asarray(inputs["Wr1"], dtype=f32)
    wf_den = np.asarray(inputs["wf_den"], dtype=f32)
    wb_den = np.asarray(inputs["wb_den"], dtype=f32)
    adj_f = np.asarray(inputs["front_sdj_den"], dtype=f32)
    sm_f = np.asarray(inputs["front_s_mask"], dtype=f32)
    adj_b = np.asarray(inputs["back_sdj_den"], dtype=f32)
    sm_b = np.asarray(inputs["back_s_mask"], dtype=f32)
    i = int(np.asarray(inputs["i"]))
    num_utter = int(np.asarray(inputs["num_utter"]))

    # ---- host-folded weights ----------------------------------------------
    v_f = (Wfk.astype(np.float64) @ wf_den[D:].astype(np.float64)).astype(f32)
    v_b = (Wbk.astype(np.float64) @ wb_den[D:].astype(np.float64)).astype(f32)
    A_f = np.vstack([Wfk, bfk[None, :]]).astype(np.float64)
    A_b = np.vstack([Wbk, bbk[None, :]]).astype(np.float64)
    G0_f = (A_f @ Wr0.astype(np.float64)).astype(f32)
    G1_f = (A_f @ Wr1.astype(np.float64)).astype(f32)
    G0_b = (A_b @ Wr0.astype(np.float64)).astype(f32)
    G1_b = (A_b @ Wr1.astype(np.float64)).astype(f32)

    # ---- host-side device layouts -----------------------------------------
    # clamp v away from 0 so the X*v / G/v fold is always well-conditioned
    def clamp(v):
        tiny = np.float32(1e-12)
        return np.where(np.abs(v) < tiny, np.where(v >= 0, tiny, -tiny), v)

    vs_f = clamp(v_f)
    vs_b = clamp(v_b)

    # X' = [K*v | k1*v | 1 | 0 0 0] in bf16
    def pack_x(Kv, kf, vs):
        xa = np.zeros((B, N, XW), dtype=bf16)
        xa[:, :, 0:D] = (Kv * vs[0:D]).astype(bf16)
        xa[:, :, D:F] = (kf * vs[D:F]).astype(bf16)
        xa[:, :, F] = np.array(1.0, dtype=bf16)
        return xa

    xall_f = pack_x(K, front_k1, vs_f)
    xall_b = pack_x(back_K, back_k2, vs_b)

    # masks [128, 2, 2, B, 2, NT/2]: mks[p,br,j,b,h,n] = m_j(b, h*1024 + p*8 + n)
    def mask_pair(adj, sm):
        m0 = (adj * sm).astype(bf16)
        m1 = (adj * (1.0 - sm)).astype(bf16)
        return m0, m1

    m0f, m1f = mask_pair(adj_f, sm_f)
    m0b, m1b = mask_pair(adj_b, sm_b)
    mks = np.empty((128, 2, 2, B, 2, NT // 2), dtype=bf16)
    for j, m in ((0, m0f), (1, m1f)):
        mks[:, 0, j] = m.reshape(B, 2, 128, NT // 2).transpose(2, 0, 1, 3)
    for j, m in ((0, m0b), (1, m1b)):
        mks[:, 1, j] = m.reshape(B, 2, 128, NT // 2).transpose(2, 0, 1, 3)

    # G' pack [128, 4, 6, D]: rows 0-767 divided by v, chunked; row 768 apart
    gpk = np.empty((128, 4, 6, D), dtype=bf16)
    g768 = np.empty((1, 4, D), dtype=bf16)
    for gi, (G, vs) in enumerate(
        ((G0_f, vs_f), (G1_f, vs_f), (G0_b, vs_b), (G1_b, vs_b))
    ):
        Gp = (G[0:F] / vs[:, None]).astype(bf16)
        gpk[:, gi] = Gp.reshape(6, 128, D).transpose(1, 0, 2)
        g768[0, gi] = G[F].astype(bf16)
    gpkf_h = np.ascontiguousarray(gpk[:, 0:2])
    gpkb_h = np.ascontiguousarray(gpk[:, 2:4])

    nc = _get_nc()

    in_maps = []
    for c in range(NCORES):
        s = slice(c * BL, (c + 1) * BL)
        in_maps.append(
            {
                "x_f": xall_f[s],
                "x_b": xall_b[s],
                "mks": np.ascontiguousarray(mks[:, :, :, s, :]),
                "gpkf": gpkf_h,
                "gpkb": gpkb_h,
                "g768": g768,
            }
        )

    trace = os.environ.get("KERNEL_TRACE", "0") == "1"
    res = run_bass_kernel_spmd(nc, in_maps, core_ids=list(range(NCORES)), trace=trace)
    last_results = res

    front = np.concatenate([r["out_f"] for r in res.results], axis=0)
    back = np.concatenate([r["out_b"] for r in res.results], axis=0)
    if i == 0:
        front = np.zeros((B, D), dtype=f32)
    if i == num_utter - 1:
        back = np.zeros((B, D), dtype=f32)
    return (front, back)


# revision 47
# speedup vs baseline: 1.0508x; 1.0508x over previous
"""Trainium2 Bass kernel for the DialogGCN GAT-style message-passing layer.

Math notes (why this is much cheaper than the reference graph):
  Kp    = concat(K, kfeat) @ Wk + bk                    (B,N,D)
  alpha = Q@wden[:D] + Kp@wden[D:] + bden               (B,N)
  w     = softmax(alpha - (1-adj)*1e30, axis=N)
  out   = sum_n w * ((Kp@Wr0)*sm + (Kp@Wr1)*(1-sm))

* softmax is invariant to per-row constants, so the Q term, bden and the
  bk@wden[D:] constant all cancel:  w = softmax_n(X_n . v) masked, where
  X = concat(K, kfeat) and v = Wk @ wden[D:]  (folded on host).
* the output is linear in the weighted sums:
    out = (sum_n w*sm*X_n | c0) @ [Wk;bk] @ Wr0 + (sum_n w*(1-sm)*X_n | c1) @ [Wk;bk] @ Wr1
  so G0 = [Wk;bk]@Wr0 and G1 = [Wk;bk]@Wr1 are folded on host (769x512 each).
* v is folded INTO the streamed tensor on host: X' = X * v (columnwise) and
  G' = G / v (rowwise) — exact algebra. The device then computes
    s_n = rowsum(X'_n) ; p_n = exp(s_n) ; U0 = sum p*m0*[X'|1] ; U1 = ...
  followed by a tiny projection (U0@G0' + U1@G1') / P, with m0 = adj*sm,
  m1 = adj*(1-sm), P = row 768 of U (the ones column of X'; the ones column
  also shifts every score by +1, which softmax cancels).

Device-side layout tricks:
* X' is uploaded as ONE bf16 tensor [BL, N, 772] = [K*v | k1*v | 1.0 | 0 0 0]
  (772 keeps every 128-token chunk 4B/8B aligned). This halves HBM traffic,
  turns the score pass into a single DVE tensor_reduce per batch (row sums,
  eligible for packed 2x/4x modes), and the ones column makes the softmax
  denominator fall out of the same PE accumulation that computes U.
* masks and the G projection matrices are pre-swizzled to their SBUF layouts
  on host and uploaded bf16, so every DMA is a dtype-preserving HWDGE
  transfer with contiguous per-partition descriptors.

Sharding: pure data parallel over batch B=32 across 8 cores (4 rows each).
"""

import os
import sys

import numpy as np

for _p in ("/opt/trn_rl_repo", "/root/.axon_site/_ro/trn_rl_repo"):
    if os.path.isdir(_p) and _p not in sys.path:
        sys.path.insert(0, _p)

B, N, D, KD = 32, 2048, 512, 256
F = D + KD  # 768
XW = F + 4  # 772: [K | k1 | 1 | 0 0 0] -- pad keeps chunk offsets 8B aligned
NCORES = 8
BL = B // NCORES  # 4 batch rows per core
NT = 16  # free-dim token tiles per batch (N = 128 * NT)

_BUILD_CACHE = {}
last_results = None  # BassKernelResults of the most recent run (for test.py)


def _build():
    """Trace the Bass program (same NEFF runs SPMD on all 8 cores)."""
    import concourse.bass as bass
    import concourse.tile as tile
    from concourse import bacc, mybir
    from concourse.masks import make_identity

    f32 = mybir.dt.float32
    bf16 = mybir.dt.bfloat16

    nc = bacc.Bacc()

    # ---- DRAM I/O ----------------------------------------------------------
    x_f = nc.dram_tensor("x_f", [BL, N, XW], bf16, kind="ExternalInput")
    x_b = nc.dram_tensor("x_b", [BL, N, XW], bf16, kind="ExternalInput")
    # masks pre-swizzled: [partition, branch, m0/m1, batch, half, n]
    mks = nc.dram_tensor("mks", [128, 2, 2, BL, 2, NT // 2], bf16, kind="ExternalInput")
    # G matrices pre-swizzled: [partition, (G0,G1), chunk, D] per branch
    gpkf = nc.dram_tensor("gpkf", [128, 2, 6, D], bf16, kind="ExternalInput")
    gpkb = nc.dram_tensor("gpkb", [128, 2, 6, D], bf16, kind="ExternalInput")
    # row 768 of each G (the bias row)
    g768 = nc.dram_tensor("g768", [1, 4, D], bf16, kind="ExternalInput")
    out_f = nc.dram_tensor("out_f", [BL, D], f32, kind="ExternalOutput")
    out_b = nc.dram_tensor("out_b", [BL, D], f32, kind="ExternalOutput")

    with tile.TileContext(nc) as tc:
        with (
            tc.tile_pool(name="singles", bufs=1) as singles,
            tc.tile_pool(name="xp", bufs=8) as xp,
            tc.tile_pool(name="scp", bufs=3) as scp,
            tc.tile_pool(name="ppp", bufs=4) as ppp,
            tc.tile_pool(name="finp", bufs=2) as finp,
            tc.tile_pool(name="psA", bufs=2, space="PSUM") as psA,
            tc.tile_pool(name="psB", bufs=2, space="PSUM") as psB,
            tc.tile_pool(name="psTr", bufs=2, space="PSUM") as psTr,
            tc.tile_pool(name="psOut", bufs=1, space="PSUM") as psOut,
        ):
            # ---- X loads, one DMA per half-batch (1.58 MB) so scoring can
            # overlap each batch's own transfer; first 8 issued upfront, the
            # rest as compute iterations free their buffers
            NH = NT // 2  # 8 chunks per half
            NGH = 2 * BL * 2  # 16 half-batches
            xsrcs = (x_f, x_b)
            xtiles = {}

            def emit_xdma(gh):
                ibr, rem = divmod(gh, 2 * BL)
                b, h = divmod(rem, 2)
                x = xp.tile([128, NH, XW], bf16, tag="x")
                nc.sync.dma_start(
                    out=x,
                    in_=xsrcs[ibr][b, h * 1024 : (h + 1) * 1024].rearrange(
                        "(p n) d -> p n d", n=NH
                    ),
                )
                xtiles[gh] = x

            for gh in range(8):
                emit_xdma(gh)

            # ---- one-time setup -------------------------------------------
            ident = singles.tile([8, 8], f32)
            make_identity(nc, ident)
            ones11 = singles.tile([1, 1], bf16)
            nc.vector.memset(ones11, 1.0)
            negone = singles.tile([128, 1], f32)
            nc.vector.memset(negone, -1.0)

            mkt = singles.tile([128, 2, 2, BL, 2, NT // 2], bf16)
            nc.scalar.dma_start(out=mkt, in_=mks[:, :, :, :, :, :])
            # branch-f G loads early; branch-b G is only consumed at the very
            # end, so its DMA is issued AFTER the last X transfer (below) to
            # keep it out of the X stream's HBM bandwidth
            gtf = singles.tile([128, 2, 6, D], bf16)
            nc.scalar.dma_start(out=gtf, in_=gpkf[:, :, :, :])
            gtb = singles.tile([128, 2, 6, D], bf16)
            g768t = singles.tile([1, 4, D], bf16)
            nc.scalar.dma_start(out=g768t, in_=g768[:, :, :])

            # ---- streaming + finishing per branch -------------------------
            for ibr, (xsrc, osrc) in enumerate(((x_f, out_f), (x_b, out_b))):
                psAt = psA.tile([8, D], f32)       # rows 0-3: U0(b) K-part, 4-7: U1(b)
                psBt = psB.tile([8, KD + 4], f32)  # cols 0:KD k1-part, col KD = P, pad

                for b in range(BL):
                  for h in range(2):
                    gh = ibr * 2 * BL + b * 2 + h
                    x = xtiles[gh]
                    NA = 3  # chunks of this half scored on ACT; rest on DVE

                    # scores, split across the idle ACT engine (one Copy-accum
                    # per chunk) and the DVE (one batched reduce)
                    sA0 = scp.tile([128, NA], f32, tag="sA0")
                    scr = scp.tile([128, XW], bf16, tag="scr")
                    for n in range(NA):
                        nc.scalar.activation(
                            out=scr,
                            in_=x[:, n, :],
                            func=mybir.ActivationFunctionType.Copy,
                            accum_out=sA0[:, n : n + 1],
                        )
                    sA1 = scp.tile([128, NH - NA], f32, tag="sA1")
                    nc.vector.tensor_reduce(
                        out=sA1,
                        in_=x[:, NA:NH, :],
                        axis=mybir.AxisListType.X,
                        op=mybir.AluOpType.add,
                    )
                    p0 = scp.tile([128, NA], bf16, tag="p0")
                    p1 = scp.tile([128, NH - NA], bf16, tag="p1")
                    # bias=-1 removes the constant from the ones column
                    nc.scalar.activation(
                        out=p0, in_=sA0, func=mybir.ActivationFunctionType.Exp,
                        bias=negone,
                    )
                    nc.scalar.activation(
                        out=p1, in_=sA1, func=mybir.ActivationFunctionType.Exp,
                        bias=negone,
                    )

                    # pp[:, n, :]: col b = p*m0, col 4+b = p*m1, rest 0
                    pp = ppp.tile([128, NH, 8], bf16, tag="pp")
                    nc.gpsimd.memset(pp, 0.0)
                    for ph, lo, hi in ((p0, 0, NA), (p1, NA, NH)):
                        nc.gpsimd.tensor_mul(
                            pp[:, lo:hi, b], ph, mkt[:, ibr, 0, b, h, lo:hi]
                        )
                        nc.gpsimd.tensor_mul(
                            pp[:, lo:hi, 4 + b], ph, mkt[:, ibr, 1, b, h, lo:hi]
                        )

                    for n in range(NH):
                        first = gh % (2 * BL) == 0 and n == 0
                        last = gh % (2 * BL) == 2 * BL - 1 and n == NH - 1
                        nc.tensor.matmul(
                            psAt, pp[:, n, :], x[:, n, 0:D], start=first, stop=last
                        )
                        nc.tensor.matmul(
                            psBt, pp[:, n, :], x[:, n, D:XW], start=first, stop=last
                        )

                    if gh + 8 < NGH:
                        emit_xdma(gh + 8)
                        if gh + 8 == NGH - 1:
                            # last X issued: queue the branch-b G load behind it
                            nc.sync.dma_start(out=gtb, in_=gpkb[:, :, :, :])

                # ---- finishing: out = (U0@G0 + U1@G1) / P ------------------
                uall = finp.tile([8, F + 1], f32, tag="uall")
                nc.vector.tensor_copy(uall[:, 0:D], psAt)
                nc.vector.tensor_copy(uall[:, D : F + 1], psBt[:, 0 : KD + 1])

                uallT = finp.tile([128, 7, 8], f32, tag="uallT")
                for k in range(6):
                    trp = psTr.tile([128, 8], f32, tag="trp")
                    nc.tensor.transpose(trp, uall[:, k * 128 : (k + 1) * 128], ident)
                    nc.vector.tensor_copy(uallT[:, k, :], trp)
                trp = psTr.tile([128, 8], f32, tag="trp")
                nc.tensor.transpose(trp[0:1, :], uall[:, F : F + 1], ident)
                nc.vector.tensor_copy(uallT[0:1, 6, :], trp[0:1, :])
                uTb = finp.tile([128, 7, 8], bf16, tag="uTb")
                nc.vector.tensor_copy(uTb, uallT)

                po = psOut.tile([4, D + 1], f32)  # cols 0:D main, col D = P (bank 2)
                gt = gtf if ibr == 0 else gtb
                for k in range(6):
                    nc.tensor.matmul(
                        po[:, 0:D], uTb[:, k, 0:4], gt[:, 0, k, :],
                        start=(k == 0), stop=False,
                    )
                nc.tensor.matmul(
                    po[:, 0:D], uTb[0:1, 6, 0:4], g768t[0:1, 2 * ibr, :],
                    start=False, stop=False,
                )
                for k in range(6):
                    nc.tensor.matmul(
                        po[:, 0:D], uTb[:, k, 4:8], gt[:, 1, k, :],
                        start=False, stop=False,
                    )
                nc.tensor.matmul(
                    po[:, 0:D], uTb[0:1, 6, 4:8], g768t[0:1, 2 * ibr + 1, :],
                    start=False, stop=True,
                )
                nc.tensor.matmul(
                    po[:, D : D + 1], uTb[0:1, 6, 0:4], ones11, start=True, stop=False
                )
                nc.tensor.matmul(
                    po[:, D : D + 1], uTb[0:1, 6, 4:8], ones11, start=False, stop=True
                )

                rp = finp.tile([4, 1], f32, tag="rp")
                nc.vector.reciprocal(rp, po[:, D : D + 1])
                osb = finp.tile([4, D], f32, tag="osb")
                nc.vector.tensor_scalar_mul(out=osb, in0=po[:, 0:D], scalar1=rp)
                nc.sync.dma_start(out=osrc[:, :], in_=osb)

    nc.compile()
    return nc


def _get_nc():
    if "nc" not in _BUILD_CACHE:
        _BUILD_CACHE["nc"] = _build()
    return _BUILD_CACHE["nc"]


def kernel(**inputs) -> tuple:
    global last_results
    from concourse import mybir
    from concourse.bass_utils import run_bass_kernel_spmd

    f32 = np.float32
    bf16 = np.dtype(mybir.dt.np(mybir.dt.bfloat16))

    K = np.asarray(inputs["K"], dtype=f32)
    front_k1 = np.asarray(inputs["front_k1"], dtype=f32)
    back_K = np.asarray(inputs["back_K"], dtype=f32)
    back_k2 = np.asarray(inputs["back_k2"], dtype=f32)
    Wfk = np.asarray(inputs["Wfk"], dtype=f32)
    bfk = np.asarray(inputs["bfk"], dtype=f32)
    Wbk = np.asarray(inputs["Wbk"], dtype=f32)
    bbk = np.asarray(inputs["bbk"], dtype=f32)
    Wr0 = np.asarray(inputs["Wr0"], dtype=f32)
    Wr1 = np.asarray(inputs["Wr1"], dtype=f32)
    wf_den = np.asarray(inputs["wf_den"], dtype=f32)
    wb_den = np.asarray(inputs["wb_den"], dtype=f32)
    adj_f = np.asarray(inputs["front_sdj_den"], dtype=f32)
    sm_f = np.asarray(inputs["front_s_mask"], dtype=f32)
    adj_b = np.asarray(inputs["back_sdj_den"], dtype=f32)
    sm_b = np.asarray(inputs["back_s_mask"], dtype=f32)
    i = int(np.asarray(inputs["i"]))
    num_utter = int(np.asarray(inputs["num_utter"]))

    # ---- host-folded weights ----------------------------------------------
    v_f = (Wfk.astype(np.float64) @ wf_den[D:].astype(np.float64)).astype(f32)
    v_b = (Wbk.astype(np.float64) @ wb_den[D:].astype(np.float64)).astype(f32)
    A_f = np.vstack([Wfk, bfk[None, :]]).astype(np.float64)
    A_b = np.vstack([Wbk, bbk[None, :]]).astype(np.float64)
    G0_f = (A_f @ Wr0.astype(np.float64)).astype(f32)
    G1_f = (A_f @ Wr1.astype(np.float64)).astype(f32)
    G0_b = (A_b @ Wr0.astype(np.float64)).astype(f32)
    G1_b = (A_b @ Wr1.astype(np.float64)).astype(f32)

    # ---- host-side device layouts -----------------------------------------
    # clamp v away from 0 so the X*v / G/v fold is always well-conditioned
    def clamp(v):
        tiny = np.float32(1e-12)
        return np.where(np.abs(v) < tiny, np.where(v >= 0, tiny, -tiny), v)

    vs_f = clamp(v_f)
    vs_b = clamp(v_b)

    # X' = [K*v | k1*v | 1 | 0 0 0] in bf16
    def pack_x(Kv, kf, vs):
        xa = np.zeros((B, N, XW), dtype=bf16)
        xa[:, :, 0:D] = (Kv * vs[0:D]).astype(bf16)
        xa[:, :, D:F] = (kf * vs[D:F]).astype(bf16)
        xa[:, :, F] = np.array(1.0, dtype=bf16)
        return xa

    xall_f = pack_x(K, front_k1, vs_f)
    xall_b = pack_x(back_K, back_k2, vs_b)

    # masks [128, 2, 2, B, 2, NT/2]: mks[p,br,j,b,h,n] = m_j(b, h*1024 + p*8 + n)
    def mask_pair(adj, sm):
        m0 = (adj * sm).astype(bf16)
        m1 = (adj * (1.0 - sm)).astype(bf16)
        return m0, m1

    m0f, m1f = mask_pair(adj_f, sm_f)
    m0b, m1b = mask_pair(adj_b, sm_b)
    mks = np.empty((128, 2, 2, B, 2, NT // 2), dtype=bf16)
    for j, m in ((0, m0f), (1, m1f)):
        mks[:, 0, j] = m.reshape(B, 2, 128, NT // 2).transpose(2, 0, 1, 3)
    for j, m in ((0, m0b), (1, m1b)):
        mks[:, 1, j] = m.reshape(B, 2, 128, NT // 2).transpose(2, 0, 1, 3)

    # G' pack [128, 4, 6, D]: rows 0-767 divided by v, chunked; row 768 apart
    gpk = np.empty((128, 4, 6, D), dtype=bf16)
    g768 = np.empty((1, 4, D), dtype=bf16)
    for gi, (G, vs) in enumerate(
        ((G0_f, vs_f), (G1_f, vs_f), (G0_b, vs_b), (G1_b, vs_b))
    ):
        Gp = (G[0:F] / vs[:, None]).astype(bf16)
        gpk[:, gi] = Gp.reshape(6, 128, D).transpose(1, 0, 2)
        g768[0, gi] = G[F].astype(bf16)
    gpkf_h = np.ascontiguousarray(gpk[:, 0:2])
    gpkb_h = np.ascontiguousarray(gpk[:, 2:4])

    nc = _get_nc()

    in_maps = []
    for c in range(NCORES):
        s = slice(c * BL, (c + 1) * BL)
        in_maps.append(
            {
                "x_f": xall_f[s],
                "x_b": xall_b[s],
                "mks": np.ascontiguousarray(mks[:, :, :, s, :]),
                "gpkf": gpkf_h,
                "gpkb": gpkb_h,
                "g768": g768,
            }
        )

    trace = os.environ.get("KERNEL_TRACE", "0") == "1"
    res = run_bass_kernel_spmd(nc, in_maps, core_ids=list(range(NCORES)), trace=trace)
    last_results = res

    front = np.concatenate([r["out_f"] for r in res.results], axis=0)
    back = np.concatenate([r["out_b"] for r in res.results], axis=0)
    if i == 0:
        front = np.zeros((B, D), dtype=f32)
    if i == num_utter - 1:
        back = np.zeros((B, D), dtype=f32)
    return (front, back)
